# revision 10
# baseline (speedup 1.0000x reference)
"""CQT magnitude kernel for Trainium2 (8 NeuronCores, Bass/Tile).

Strategy (v2: symmetry-folded)
------------------------------
The CQT filterbank is exactly symmetric about its center tap: kr (Hann*cos)
is even, ki (Hann*sin) is odd.  Folding the contraction halves PE work:

    re[k,n] = sum_u kr_f[k,u] * s[n,u],   s[n,u] = x[nH+c+u] + x[nH+c-u]
    im[k,n] = sum_u ki_f[k,u] * d[n,u],   d[n,u] = x[nH+c+u] - x[nH+c-u]

with u in [0, L/2), kr_f[k,0] halved (s[n,0] = 2*x_center).  s/d are formed
on the DVE from two SBUF-resident signal layouts: Xf[p,j] = xpad[j*128+p]
(forward) and Xr[p,j] = xpad[(j+1)*128-p] (partition-reversed), so that for
u-chunk c, frame n:  x[+u] = Xf[p, 271+c+4n],  x[-u] = Xr[p, 270-c+4n].

Work: 271 m0 u-chunks (bins 0..127) + 24 m1 u-chunks (bins 128..251) + 1 pad
= 296 units, 37 per core.  Per unit: one DVE add + one sub ([128,516] bf16)
and 4 PE matmuls (re/im x 2 track-pairs, N=258) accumulating in 8 PSUM banks
(class A = slots 0..11 -> banks 0..3, class B = slots 12..36 -> banks 4..7).
Each (core, class) cell holds chunks of a single bin-block; the host maps
cell partials to bins: cores 0-5 all m0; cores 6,7 class A = m1 halves.

Flushes: Act-engine copies PSUM->SBUF (fp16), one DMA per class.  The host
sums the 16 cell partials and takes sqrt(re^2+im^2).
"""

import numpy as np

# ---- problem constants (hardcoded per contract) ----
SR_B, SR_TR, SR_T = 2, 2, 65536        # x shape
NTRACKS = SR_B * SR_TR                 # 4
KBINS = 252
L = 69376                              # filterbank window length
HL = L // 2                            # 34688 folded taps
HOP = 512
PCH = 128
NCH0 = 271                             # m0 u-chunks (bins 0..127)
NCH1 = 24                              # m1 u-chunks (bins 128..251)
NF = 1 + SR_T // HOP                   # 129 frames
N2 = 2 * NF                            # 258 matmul columns (2 tracks)
NCORES = 8

NSLOTS = 37                            # per-core units (296 = 8*37 total)
NA = 12                                # class-A slots -> PSUM banks 0-3
NB = NSLOTS - NA                       # 25 class-B slots -> banks 4-7
FCA = NA + 4 * (NF - 1)                # 524 forward cols, class A window
FCB = NB + 4 * (NF - 1)                # 537 forward cols, class B window
XOFF = [0, FCA, FCA + FCB, 2 * FCA + FCB]   # FA | FB | RA | RB col starts
XCOLS = 2 * (FCA + FCB)                # 2122

GSIZES = [2, 4, 4, 4, 4, 4, 4, 4, 4, 3]   # kt DMA groups (sum = 37)
NG = len(GSIZES)
_GOF = np.cumsum([0] + GSIZES).tolist()


def _slot_group(s):
    for g in range(NG):
        if s < _GOF[g + 1]:
            return g, s - _GOF[g]
    raise ValueError(s)


# per-core chunk assignment: (classA_start, classB_start, classA_is_m1)
def _core_assign(q):
    if q < 6:
        return 37 * q, 37 * q + 12, False
    if q == 6:
        return 0, 222, True
    return 12, 247, True                # core 7; B covers 247..270 + 1 pad


_PROG = None


def _build_program():
    import concourse.bass as bass
    import concourse.mybir as mybir
    from concourse import bacc
    from concourse.tile import TileContext

    f32 = mybir.dt.float32
    f16 = mybir.dt.float16
    bf16 = mybir.dt.bfloat16
    COPY = mybir.ActivationFunctionType.Copy

    nc = bacc.Bacc(None, name="cqt_fold")
    x_d = nc.dram_tensor("x", [128, XCOLS, 4], bf16, kind="ExternalInput")
    kt_d = nc.dram_tensor("kt", [NG, 128, 4 * 256], bf16, kind="ExternalInput")
    out_d = nc.dram_tensor("out", [2, 128, 4 * N2], f16, kind="ExternalOutput")

    def vw(t, off, pat):
        return bass.AP(tensor=t.tensor, offset=t.offset + off, ap=[t.ap[0]] + pat)

    with TileContext(nc) as tc:
        with (
            tc.tile_pool(name="xp", bufs=1) as xp,
            tc.tile_pool(name="ktp", bufs=NG) as ktp,
            tc.tile_pool(name="sp", bufs=3) as sp,
            tc.tile_pool(name="dp", bufs=3) as dp,
            tc.tile_pool(name="wp", bufs=1) as wp,
            tc.tile_pool(name="accp", bufs=1, space="PSUM") as accp,
        ):
            fa_t = xp.tile([128, FCA, 4], bf16, name="fa")
            fb_t = xp.tile([128, FCB, 4], bf16, name="fb")
            ra_t = xp.tile([128, FCA, 4], bf16, name="ra")
            rb_t = xp.tile([128, FCB, 4], bf16, name="rb")

            kts = [
                ktp.tile([128, 4 * 256], bf16, tag="kt", name=f"kt{g}")
                for g in range(NG)
            ]

            # DMA issues spread across engine queues so transfers start in
            # parallel; critical path (fa, ra, ktg0) first on each queue.
            nc.sync.dma_start(out=fa_t, in_=x_d[:, XOFF[0]:XOFF[0] + FCA, :])
            nc.gpsimd.dma_start(out=ra_t, in_=x_d[:, XOFF[2]:XOFF[2] + FCA, :])
            for g in range(NG):
                nc.scalar.dma_start(
                    out=kts[g][:, : GSIZES[g] * 256],
                    in_=kt_d[g, :, : GSIZES[g] * 256],
                )
            nc.sync.dma_start(out=fb_t, in_=x_d[:, XOFF[1]:XOFF[1] + FCB, :])
            nc.sync.dma_start(out=rb_t, in_=x_d[:, XOFF[3]:XOFF[3] + FCB, :])

            accs = [
                accp.tile([128, N2], f32, tag=f"acc{b}", name=f"acc{b}")
                for b in range(8)
            ]
            sta = wp.tile([128, 4 * N2], f16, name="sta")
            stb = wp.tile([128, 4 * N2], f16, name="stb")

            FRAME_AP = [[16, NF], [1, 4]]      # frames x 4 tracks on x tiles
            for s in range(NSLOTS):
                cls = 0 if s < NA else 1
                if cls == 0:
                    f_t, r_t = fa_t, ra_t
                    f_off, r_off = 4 * s, 4 * (NA - 1 - s)
                else:
                    f_t, r_t = fb_t, rb_t
                    f_off, r_off = 4 * (s - NA), 4 * (NSLOTS - 1 - s)
                first = s == 0 or s == NA
                last = s == NA - 1 or s == NSLOTS - 1

                s_t = sp.tile([128, 4 * NF], bf16, tag="s")
                d_t = dp.tile([128, 4 * NF], bf16, tag="d")
                f_ap = vw(f_t, f_off, FRAME_AP)
                r_ap = vw(r_t, r_off, FRAME_AP)
                nc.vector.tensor_add(vw(s_t, 0, [[4, NF], [1, 4]]), f_ap, r_ap)
                nc.gpsimd.tensor_sub(vw(d_t, 0, [[4, NF], [1, 4]]), f_ap, r_ap)

                g, si = _slot_group(s)
                for part in range(2):
                    lhsT = kts[g][:, si * 256 + part * 128: si * 256 + (part + 1) * 128]
                    src = s_t if part == 0 else d_t
                    for tp in range(2):
                        rhs = vw(src, 2 * tp, [[4, NF], [1, 2]])
                        nc.tensor.matmul(
                            accs[cls * 4 + part * 2 + tp],
                            lhsT,
                            rhs,
                            start=first,
                            stop=last,
                        )

                if s == NA - 1:
                    for b in range(4):
                        nc.scalar.activation(
                            sta[:, b * N2:(b + 1) * N2], accs[b], COPY
                        )
                    nc.scalar.dma_start(out=out_d[0, :, :], in_=sta)

            # tail flush: split copies across the idle Act + Vector engines
            nc.scalar.activation(stb[:, 0 * N2:1 * N2], accs[4], COPY)
            nc.scalar.activation(stb[:, 1 * N2:2 * N2], accs[5], COPY)
            nc.vector.tensor_copy(stb[:, 2 * N2:3 * N2], accs[6])
            nc.vector.tensor_copy(stb[:, 3 * N2:4 * N2], accs[7])
            nc.scalar.dma_start(out=out_d[1, :, :], in_=stb)
    nc.finalize()
    return nc


def _take_cols(src, start, n):
    """src[:, start:start+n, :] with zero padding outside src's col range."""
    out = np.zeros((128, n, 4), np.float32)
    s0, s1 = max(0, start), min(src.shape[1], start + n)
    if s1 > s0:
        out[:, s0 - start:s1 - start] = src[:, s0:s1]
    return out


def _pack_inputs(x, kr, ki):
    import ml_dtypes

    bf16 = ml_dtypes.bfloat16
    xf = np.ascontiguousarray(np.asarray(x, dtype=np.float32).reshape(NTRACKS, SR_T))
    kr = np.asarray(kr, dtype=np.float32)
    ki = np.asarray(ki, dtype=np.float32)

    xpad = np.zeros((NTRACKS, HL + SR_T + HL), np.float32)
    xpad[:, HL:HL + SR_T] = xf
    A = xpad.reshape(NTRACKS, -1, PCH)            # [4, 1054, 128]
    Xf = np.ascontiguousarray(A.transpose(2, 1, 0))     # [128, 1054, 4]
    NR = 784                                       # reversed cols needed <= 783
    Xr = np.zeros((128, NR, 4), np.float32)
    Xr[0] = A[:, 1:NR + 1, 0].transpose(1, 0)      # p=0: xpad[(j+1)*128]
    Xr[1:] = np.flip(A[:, :NR, 1:], axis=2).transpose(2, 1, 0)

    krF = kr[:, HL:].copy()
    krF[:, 0] *= 0.5                               # s[n,0] double-counts center
    kiF = ki[:, HL:]

    in_maps = []
    for q in range(NCORES):
        cA0, cB0, a_m1 = _core_assign(q)
        fa = _take_cols(Xf, 271 + cA0, FCA)
        fb = _take_cols(Xf, 271 + cB0, FCB)
        ra = _take_cols(Xr, 270 - cA0 - (NA - 1), FCA)
        rb = _take_cols(Xr, 270 - cB0 - (NB - 1), FCB)
        xarr = np.ascontiguousarray(
            np.concatenate([fa, fb, ra, rb], axis=1).astype(bf16)
        )

        kt_all = np.zeros((NG, 128, 4 * 256), np.float32)
        for s in range(NSLOTS):
            m1 = a_m1 and s < NA
            c = cA0 + s if s < NA else cB0 + (s - NA)
            if c * PCH >= HL:
                continue                           # pad slot: zero weights
            rows = slice(128, KBINS) if m1 else slice(0, 128)
            g, si = _slot_group(s)
            for part, mat in ((0, krF), (1, kiF)):
                blk = mat[rows, c * PCH:(c + 1) * PCH]     # [nbins, 128]
                buf = np.zeros((128, PCH), np.float32)
                buf[: blk.shape[0]] = blk
                kt_all[g, :, si * 256 + part * 128: si * 256 + (part + 1) * 128] = buf.T
        in_maps.append({
            "x": xarr,
            "kt": np.ascontiguousarray(kt_all.astype(bf16)),
        })
    return in_maps


def _combine(outs):
    re_acc = np.zeros((KBINS, NTRACKS, NF), np.float32)
    im_acc = np.zeros((KBINS, NTRACKS, NF), np.float32)
    for q in range(NCORES):
        _, _, a_m1 = _core_assign(q)
        out = np.asarray(outs[q], dtype=np.float32)   # [2, 128, 4*258]
        for b in range(8):
            cls, part, tp = b >> 2, (b >> 1) & 1, b & 1
            m1 = a_m1 and cls == 0
            arr = out[cls, :, (b & 3) * N2:((b & 3) + 1) * N2].reshape(128, NF, 2)
            rows = slice(128, KBINS) if m1 else slice(0, 128)
            nrows = KBINS - 128 if m1 else 128
            tgt = re_acc if part == 0 else im_acc
            tgt[rows, tp * 2:(tp + 1) * 2] += arr[:nrows].transpose(0, 2, 1)
    y = np.sqrt(re_acc**2 + im_acc**2)                # [252, 4, 129]
    return np.ascontiguousarray(
        y.reshape(KBINS, SR_B, SR_TR, NF).transpose(1, 0, 3, 2)
    )


def kernel(x, kr, ki):
    global _PROG
    from concourse.bass_utils import run_bass_kernel_spmd

    if _PROG is None:
        _PROG = _build_program()
    in_maps = _pack_inputs(x, kr, ki)
    res = run_bass_kernel_spmd(_PROG, in_maps, core_ids=list(range(NCORES)))
    outs = [res.results[q]["out"] for q in range(NCORES)]
    return _combine(outs)


# revision 17
# speedup vs baseline: 1.5803x; 1.5803x over previous
"""CQT magnitude kernel for Trainium2 (8 NeuronCores, Bass/Tile).

Strategy (v2: symmetry-folded)
------------------------------
The CQT filterbank is exactly symmetric about its center tap: kr (Hann*cos)
is even, ki (Hann*sin) is odd.  Folding the contraction halves PE work:

    re[k,n] = sum_u kr_f[k,u] * s[n,u],   s[n,u] = x[nH+c+u] + x[nH+c-u]
    im[k,n] = sum_u ki_f[k,u] * d[n,u],   d[n,u] = x[nH+c+u] - x[nH+c-u]

with u in [0, L/2), kr_f[k,0] halved (s[n,0] = 2*x_center).  s/d are formed
on the DVE from two SBUF-resident signal layouts: Xf[p,j] = xpad[j*128+p]
(forward) and Xr[p,j] = xpad[(j+1)*128-p] (partition-reversed), so that for
u-chunk c, frame n:  x[+u] = Xf[p, 271+c+4n],  x[-u] = Xr[p, 270-c+4n].

Work: 271 m0 u-chunks (bins 0..127) + 24 m1 u-chunks (bins 128..251) + 1 pad
= 296 units, 37 per core.  Per unit: one DVE add + one sub ([128,516] bf16)
and 4 PE matmuls (re/im x 2 track-pairs, N=258) accumulating in 8 PSUM banks
(class A = slots 0..11 -> banks 0..3, class B = slots 12..36 -> banks 4..7).
Each (core, class) cell holds chunks of a single bin-block; the host maps
cell partials to bins: cores 0-5 all m0; cores 6,7 class A = m1 halves.

Flushes: Act-engine copies PSUM->SBUF (fp16), one DMA per class.  The host
sums the 16 cell partials and takes sqrt(re^2+im^2).
"""

import numpy as np

# ---- problem constants (hardcoded per contract) ----
SR_B, SR_TR, SR_T = 2, 2, 65536        # x shape
NTRACKS = SR_B * SR_TR                 # 4
KBINS = 252
L = 69376                              # filterbank window length
HL = L // 2                            # 34688 folded taps
HOP = 512
PCH = 128
NCH0 = 271                             # m0 u-chunks (bins 0..127)
NCH1 = 24                              # m1 u-chunks (bins 128..251)
NF = 1 + SR_T // HOP                   # 129 frames
N2 = 2 * NF                            # 258 matmul columns (2 tracks)
NCORES = 8

NSLOTS = 37                            # per-core units (296 = 8*37 total)
NA = 12                                # class-A slots -> PSUM banks 0-3
NB = NSLOTS - NA                       # 25 class-B slots -> banks 4-7
FCA = NA + 4 * (NF - 1)                # 524 forward cols, class A window
FCB = NB + 4 * (NF - 1)                # 537 forward cols, class B window
# x blocks: FA | FB | RA+ | RA- | RB+ | RB-  (R shipped twice: +r and -r, so
# one stride-0-broadcast DVE add per slot yields both s = f+r and d = f-r)
XOFF = [0, FCA, FCA + FCB, 2 * FCA + FCB, 3 * FCA + FCB, 3 * FCA + 2 * FCB]
XCOLS = 3 * (FCA + FCB)                # 3183

GSIZES = [2, 4, 4, 4, 4, 4, 4, 4, 4, 3]   # kt DMA groups (sum = 37)
NG = len(GSIZES)
_GOF = np.cumsum([0] + GSIZES).tolist()


def _slot_group(s):
    for g in range(NG):
        if s < _GOF[g + 1]:
            return g, s - _GOF[g]
    raise ValueError(s)


# per-core chunk assignment: (classA_start, classB_start, classA_is_m1)
def _core_assign(q):
    if q < 6:
        return 37 * q, 37 * q + 12, False
    if q == 6:
        return 0, 222, True
    return 12, 247, True                # core 7; B covers 247..270 + 1 pad


_PROG = None


def _build_program():
    import concourse.bass as bass
    import concourse.mybir as mybir
    from concourse import bacc
    from concourse.tile import TileContext

    f32 = mybir.dt.float32
    f16 = mybir.dt.float16
    bf16 = mybir.dt.bfloat16
    COPY = mybir.ActivationFunctionType.Copy

    nc = bacc.Bacc(None, name="cqt_fold")
    x_d = nc.dram_tensor("x", [128, XCOLS, 4], bf16, kind="ExternalInput")
    kt_d = nc.dram_tensor("kt", [NG, 128, 4 * 256], bf16, kind="ExternalInput")
    out_d = nc.dram_tensor("out", [2, 128, 4 * N2], f16, kind="ExternalOutput")

    def vw(t, off, pat):
        return bass.AP(tensor=t.tensor, offset=t.offset + off, ap=[t.ap[0]] + pat)

    with TileContext(nc) as tc:
        with (
            tc.tile_pool(name="xp", bufs=1) as xp,
            tc.tile_pool(name="ktp", bufs=NG) as ktp,
            tc.tile_pool(name="sp", bufs=4) as sp,
            tc.tile_pool(name="wp", bufs=1) as wp,
            tc.tile_pool(name="accp", bufs=1, space="PSUM") as accp,
        ):
            fa_t = xp.tile([128, FCA, 4], bf16, name="fa")
            fb_t = xp.tile([128, FCB, 4], bf16, name="fb")
            ra_t = xp.tile([128, 2 * FCA, 4], bf16, name="ra")   # [+r | -r]
            rb_t = xp.tile([128, 2 * FCB, 4], bf16, name="rb")

            kts = [
                ktp.tile([128, 4 * 256], bf16, tag="kt", name=f"kt{g}")
                for g in range(NG)
            ]

            # DMA issues spread across engine queues so transfers start in
            # parallel; critical path (fa, ra, ktg0) first on each queue.
            nc.sync.dma_start(out=fa_t, in_=x_d[:, XOFF[0]:XOFF[0] + FCA, :])
            nc.gpsimd.dma_start(
                out=ra_t, in_=x_d[:, XOFF[2]:XOFF[2] + 2 * FCA, :]
            )
            for g in range(NG):
                nc.scalar.dma_start(
                    out=kts[g][:, : GSIZES[g] * 256],
                    in_=kt_d[g, :, : GSIZES[g] * 256],
                )
            nc.sync.dma_start(out=fb_t, in_=x_d[:, XOFF[1]:XOFF[1] + FCB, :])
            nc.sync.dma_start(
                out=rb_t, in_=x_d[:, XOFF[4]:XOFF[4] + 2 * FCB, :]
            )

            accs = [
                accp.tile([128, N2], f32, tag=f"acc{b}", name=f"acc{b}")
                for b in range(8)
            ]
            sta = wp.tile([128, 4 * N2], f16, name="sta")
            stb = wp.tile([128, 4 * N2], f16, name="stb")

            for s in range(NSLOTS):
                cls = 0 if s < NA else 1
                if cls == 0:
                    f_t, r_t, rw = fa_t, ra_t, FCA
                    f_off, r_off = 4 * s, 4 * (NA - 1 - s)
                else:
                    f_t, r_t, rw = fb_t, rb_t, FCB
                    f_off, r_off = 4 * (s - NA), 4 * (NSLOTS - 1 - s)
                first = s == 0 or s == NA
                last = s == NA - 1 or s == NSLOTS - 1

                # one DVE op -> [s | d] stacked: ver 0 adds +r, ver 1 adds -r
                sd_t = sp.tile([128, 8 * NF], bf16, tag="sd")
                f_ap = vw(f_t, f_off, [[0, 2], [16, NF], [1, 4]])
                r_ap = vw(r_t, r_off, [[4 * rw, 2], [16, NF], [1, 4]])
                nc.vector.tensor_add(
                    vw(sd_t, 0, [[4 * NF, 2], [4, NF], [1, 4]]), f_ap, r_ap
                )

                g, si = _slot_group(s)
                for part in range(2):
                    lhsT = kts[g][:, si * 256 + part * 128: si * 256 + (part + 1) * 128]
                    for tp in range(2):
                        rhs = vw(sd_t, part * 4 * NF + 2 * tp, [[4, NF], [1, 2]])
                        nc.tensor.matmul(
                            accs[cls * 4 + part * 2 + tp],
                            lhsT,
                            rhs,
                            start=first,
                            stop=last,
                        )

                if s == NA - 1:
                    for b in range(4):
                        nc.scalar.activation(
                            sta[:, b * N2:(b + 1) * N2], accs[b], COPY
                        )
                    nc.scalar.dma_start(out=out_d[0, :, :], in_=sta)

            # tail flush: split copies across the idle Act + Vector engines
            nc.scalar.activation(stb[:, 0 * N2:1 * N2], accs[4], COPY)
            nc.scalar.activation(stb[:, 1 * N2:2 * N2], accs[5], COPY)
            nc.vector.tensor_copy(stb[:, 2 * N2:3 * N2], accs[6])
            nc.vector.tensor_copy(stb[:, 3 * N2:4 * N2], accs[7])
            nc.scalar.dma_start(out=out_d[1, :, :], in_=stb)
    nc.finalize()
    return nc


def _take_cols(src, start, n):
    """src[:, start:start+n, :] with zero padding outside src's col range."""
    out = np.zeros((128, n, 4), np.float32)
    s0, s1 = max(0, start), min(src.shape[1], start + n)
    if s1 > s0:
        out[:, s0 - start:s1 - start] = src[:, s0:s1]
    return out


def _pack_inputs(x, kr, ki):
    import ml_dtypes

    bf16 = ml_dtypes.bfloat16
    xf = np.ascontiguousarray(np.asarray(x, dtype=np.float32).reshape(NTRACKS, SR_T))
    kr = np.asarray(kr, dtype=np.float32)
    ki = np.asarray(ki, dtype=np.float32)

    xpad = np.zeros((NTRACKS, HL + SR_T + HL), np.float32)
    xpad[:, HL:HL + SR_T] = xf
    A = xpad.reshape(NTRACKS, -1, PCH)            # [4, 1054, 128]
    Xf = np.ascontiguousarray(A.transpose(2, 1, 0))     # [128, 1054, 4]
    NR = 784                                       # reversed cols needed <= 783
    Xr = np.zeros((128, NR, 4), np.float32)
    Xr[0] = A[:, 1:NR + 1, 0].transpose(1, 0)      # p=0: xpad[(j+1)*128]
    Xr[1:] = np.flip(A[:, :NR, 1:], axis=2).transpose(2, 1, 0)

    krF = kr[:, HL:].copy()
    krF[:, 0] *= 0.5                               # s[n,0] double-counts center
    kiF = ki[:, HL:]

    in_maps = []
    for q in range(NCORES):
        cA0, cB0, a_m1 = _core_assign(q)
        fa = _take_cols(Xf, 271 + cA0, FCA)
        fb = _take_cols(Xf, 271 + cB0, FCB)
        ra = _take_cols(Xr, 270 - cA0 - (NA - 1), FCA)
        rb = _take_cols(Xr, 270 - cB0 - (NB - 1), FCB)
        xarr = np.ascontiguousarray(
            np.concatenate([fa, fb, ra, -ra, rb, -rb], axis=1).astype(bf16)
        )

        kt_all = np.zeros((NG, 128, 4 * 256), np.float32)
        for s in range(NSLOTS):
            m1 = a_m1 and s < NA
            c = cA0 + s if s < NA else cB0 + (s - NA)
            if c * PCH >= HL:
                continue                           # pad slot: zero weights
            rows = slice(128, KBINS) if m1 else slice(0, 128)
            g, si = _slot_group(s)
            for part, mat in ((0, krF), (1, kiF)):
                blk = mat[rows, c * PCH:(c + 1) * PCH]     # [nbins, 128]
                buf = np.zeros((128, PCH), np.float32)
                buf[: blk.shape[0]] = blk
                kt_all[g, :, si * 256 + part * 128: si * 256 + (part + 1) * 128] = buf.T
        in_maps.append({
            "x": xarr,
            "kt": np.ascontiguousarray(kt_all.astype(bf16)),
        })
    return in_maps


def _combine(outs):
    re_acc = np.zeros((KBINS, NTRACKS, NF), np.float32)
    im_acc = np.zeros((KBINS, NTRACKS, NF), np.float32)
    for q in range(NCORES):
        _, _, a_m1 = _core_assign(q)
        out = np.asarray(outs[q], dtype=np.float32)   # [2, 128, 4*258]
        for b in range(8):
            cls, part, tp = b >> 2, (b >> 1) & 1, b & 1
            m1 = a_m1 and cls == 0
            arr = out[cls, :, (b & 3) * N2:((b & 3) + 1) * N2].reshape(128, NF, 2)
            rows = slice(128, KBINS) if m1 else slice(0, 128)
            nrows = KBINS - 128 if m1 else 128
            tgt = re_acc if part == 0 else im_acc
            tgt[rows, tp * 2:(tp + 1) * 2] += arr[:nrows].transpose(0, 2, 1)
    y = np.sqrt(re_acc**2 + im_acc**2)                # [252, 4, 129]
    return np.ascontiguousarray(
        y.reshape(KBINS, SR_B, SR_TR, NF).transpose(1, 0, 3, 2)
    )


def kernel(x, kr, ki):
    global _PROG
    from concourse.bass_utils import run_bass_kernel_spmd

    if _PROG is None:
        _PROG = _build_program()
    in_maps = _pack_inputs(x, kr, ki)
    res = run_bass_kernel_spmd(_PROG, in_maps, core_ids=list(range(NCORES)))
    outs = [res.results[q]["out"] for q in range(NCORES)]
    return _combine(outs)


# revision 19
# speedup vs baseline: 1.5930x; 1.0080x over previous
"""CQT magnitude kernel for Trainium2 (8 NeuronCores, Bass/Tile).

Strategy (v2: symmetry-folded)
------------------------------
The CQT filterbank is exactly symmetric about its center tap: kr (Hann*cos)
is even, ki (Hann*sin) is odd.  Folding the contraction halves PE work:

    re[k,n] = sum_u kr_f[k,u] * s[n,u],   s[n,u] = x[nH+c+u] + x[nH+c-u]
    im[k,n] = sum_u ki_f[k,u] * d[n,u],   d[n,u] = x[nH+c+u] - x[nH+c-u]

with u in [0, L/2), kr_f[k,0] halved (s[n,0] = 2*x_center).  s/d are formed
on the DVE from two SBUF-resident signal layouts: Xf[p,j] = xpad[j*128+p]
(forward) and Xr[p,j] = xpad[(j+1)*128-p] (partition-reversed), so that for
u-chunk c, frame n:  x[+u] = Xf[p, 271+c+4n],  x[-u] = Xr[p, 270-c+4n].

Work: 271 m0 u-chunks (bins 0..127) + 24 m1 u-chunks (bins 128..251) + 1 pad
= 296 units, 37 per core.  Per unit: one DVE add + one sub ([128,516] bf16)
and 4 PE matmuls (re/im x 2 track-pairs, N=258) accumulating in 8 PSUM banks
(class A = slots 0..11 -> banks 0..3, class B = slots 12..36 -> banks 4..7).
Each (core, class) cell holds chunks of a single bin-block; the host maps
cell partials to bins: cores 0-5 all m0; cores 6,7 class A = m1 halves.

Flushes: Act-engine copies PSUM->SBUF (fp16), one DMA per class.  The host
sums the 16 cell partials and takes sqrt(re^2+im^2).
"""

import numpy as np

# ---- problem constants (hardcoded per contract) ----
SR_B, SR_TR, SR_T = 2, 2, 65536        # x shape
NTRACKS = SR_B * SR_TR                 # 4
KBINS = 252
L = 69376                              # filterbank window length
HL = L // 2                            # 34688 folded taps
HOP = 512
PCH = 128
NCH0 = 271                             # m0 u-chunks (bins 0..127)
NCH1 = 24                              # m1 u-chunks (bins 128..251)
NF = 1 + SR_T // HOP                   # 129 frames
N2 = 2 * NF                            # 258 matmul columns (2 tracks)
NCORES = 8

NSLOTS = 37                            # per-core units (296 = 8*37 total)
NA = 12                                # class-A slots -> PSUM banks 0-3
NB = NSLOTS - NA                       # 25 class-B slots -> banks 4-7
FCA = NA + 4 * (NF - 1)                # 524 forward cols, class A window
FCB = NB + 4 * (NF - 1)                # 537 forward cols, class B window
# x blocks: FA | FB | RA+ | RA- | RB+ | RB-  (R shipped twice: +r and -r, so
# one stride-0-broadcast DVE add per slot yields both s = f+r and d = f-r)
XOFF = [0, FCA, FCA + FCB, 2 * FCA + FCB, 3 * FCA + FCB, 3 * FCA + 2 * FCB]
XCOLS = 3 * (FCA + FCB)                # 3183

GSIZES = [2, 4, 4, 4, 4, 4, 4, 4, 4, 3]   # kt DMA groups (sum = 37)
NG = len(GSIZES)
_GOF = np.cumsum([0] + GSIZES).tolist()


def _slot_group(s):
    for g in range(NG):
        if s < _GOF[g + 1]:
            return g, s - _GOF[g]
    raise ValueError(s)


# per-core chunk assignment: (classA_start, classB_start, classA_is_m1)
def _core_assign(q):
    if q < 6:
        return 37 * q, 37 * q + 12, False
    if q == 6:
        return 0, 222, True
    return 12, 247, True                # core 7; B covers 247..270 + 1 pad


_PROG = None


def _build_program():
    import concourse.bass as bass
    import concourse.mybir as mybir
    from concourse import bacc
    from concourse.tile import TileContext

    f32 = mybir.dt.float32
    f16 = mybir.dt.float16
    bf16 = mybir.dt.bfloat16
    COPY = mybir.ActivationFunctionType.Copy

    nc = bacc.Bacc(None, name="cqt_fold")
    x_d = nc.dram_tensor("x", [128, XCOLS, 4], bf16, kind="ExternalInput")
    kt_d = nc.dram_tensor("kt", [NG, 128, 4 * 256], bf16, kind="ExternalInput")
    out_d = nc.dram_tensor("out", [2, 128, 4 * N2], f16, kind="ExternalOutput")

    def vw(t, off, pat):
        return bass.AP(tensor=t.tensor, offset=t.offset + off, ap=[t.ap[0]] + pat)

    with TileContext(nc) as tc:
        with (
            tc.tile_pool(name="xp", bufs=1) as xp,
            tc.tile_pool(name="ktp", bufs=NG) as ktp,
            tc.tile_pool(name="sp", bufs=4) as sp,
            tc.tile_pool(name="wp", bufs=1) as wp,
            tc.tile_pool(name="accp", bufs=1, space="PSUM") as accp,
        ):
            fa_t = xp.tile([128, FCA, 4], bf16, name="fa")
            fb_t = xp.tile([128, FCB, 4], bf16, name="fb")
            ra_t = xp.tile([128, 2 * FCA, 4], bf16, name="ra")   # [+r | -r]
            rb_t = xp.tile([128, 2 * FCB, 4], bf16, name="rb")

            kts = [
                ktp.tile([128, 4 * 256], bf16, tag="kt", name=f"kt{g}")
                for g in range(NG)
            ]

            # DMA issues spread across engine queues so transfers start in
            # parallel; critical path (fa, ra, ktg0) first on each queue.
            # Balance the slot-0 critical transfers (fa, ra+, ra-, ktg0)
            # across the three DMA-capable queues: per-ring BW is the limit.
            nc.sync.dma_start(out=fa_t, in_=x_d[:, XOFF[0]:XOFF[0] + FCA, :])
            nc.gpsimd.dma_start(
                out=ra_t[:, :FCA], in_=x_d[:, XOFF[2]:XOFF[2] + FCA, :]
            )
            nc.scalar.dma_start(
                out=kts[0][:, : GSIZES[0] * 256],
                in_=kt_d[0, :, : GSIZES[0] * 256],
            )
            nc.scalar.dma_start(
                out=ra_t[:, FCA:], in_=x_d[:, XOFF[3]:XOFF[3] + FCA, :]
            )
            nc.sync.dma_start(out=fb_t, in_=x_d[:, XOFF[1]:XOFF[1] + FCB, :])
            nc.sync.dma_start(
                out=rb_t, in_=x_d[:, XOFF[4]:XOFF[4] + 2 * FCB, :]
            )
            for g in range(1, NG):
                nc.scalar.dma_start(
                    out=kts[g][:, : GSIZES[g] * 256],
                    in_=kt_d[g, :, : GSIZES[g] * 256],
                )

            accs = [
                accp.tile([128, N2], f32, tag=f"acc{b}", name=f"acc{b}")
                for b in range(8)
            ]
            sta = wp.tile([128, 4 * N2], f16, name="sta")
            stb = wp.tile([128, 4 * N2], f16, name="stb")

            for s in range(NSLOTS):
                cls = 0 if s < NA else 1
                if cls == 0:
                    f_t, r_t, rw = fa_t, ra_t, FCA
                    f_off, r_off = 4 * s, 4 * (NA - 1 - s)
                else:
                    f_t, r_t, rw = fb_t, rb_t, FCB
                    f_off, r_off = 4 * (s - NA), 4 * (NSLOTS - 1 - s)
                first = s == 0 or s == NA
                last = s == NA - 1 or s == NSLOTS - 1

                # one DVE op -> [s | d] stacked: ver 0 adds +r, ver 1 adds -r
                sd_t = sp.tile([128, 8 * NF], bf16, tag="sd")
                f_ap = vw(f_t, f_off, [[0, 2], [16, NF], [1, 4]])
                r_ap = vw(r_t, r_off, [[4 * rw, 2], [16, NF], [1, 4]])
                nc.vector.tensor_add(
                    vw(sd_t, 0, [[4 * NF, 2], [4, NF], [1, 4]]), f_ap, r_ap
                )

                g, si = _slot_group(s)
                for part in range(2):
                    lhsT = kts[g][:, si * 256 + part * 128: si * 256 + (part + 1) * 128]
                    for tp in range(2):
                        rhs = vw(sd_t, part * 4 * NF + 2 * tp, [[4, NF], [1, 2]])
                        nc.tensor.matmul(
                            accs[cls * 4 + part * 2 + tp],
                            lhsT,
                            rhs,
                            start=first,
                            stop=last,
                        )

                if s == NA - 1:
                    for b in range(4):
                        nc.scalar.activation(
                            sta[:, b * N2:(b + 1) * N2], accs[b], COPY
                        )
                    nc.scalar.dma_start(out=out_d[0, :, :], in_=sta)

            # tail flush: split copies across the idle Act + Vector engines
            nc.scalar.activation(stb[:, 0 * N2:1 * N2], accs[4], COPY)
            nc.scalar.activation(stb[:, 1 * N2:2 * N2], accs[5], COPY)
            nc.vector.tensor_copy(stb[:, 2 * N2:3 * N2], accs[6])
            nc.vector.tensor_copy(stb[:, 3 * N2:4 * N2], accs[7])
            nc.sync.dma_start(out=out_d[1, :, :], in_=stb)
    nc.finalize()
    return nc


def _take_cols(src, start, n):
    """src[:, start:start+n, :] with zero padding outside src's col range."""
    out = np.zeros((128, n, 4), np.float32)
    s0, s1 = max(0, start), min(src.shape[1], start + n)
    if s1 > s0:
        out[:, s0 - start:s1 - start] = src[:, s0:s1]
    return out


def _pack_inputs(x, kr, ki):
    import ml_dtypes

    bf16 = ml_dtypes.bfloat16
    xf = np.ascontiguousarray(np.asarray(x, dtype=np.float32).reshape(NTRACKS, SR_T))
    kr = np.asarray(kr, dtype=np.float32)
    ki = np.asarray(ki, dtype=np.float32)

    xpad = np.zeros((NTRACKS, HL + SR_T + HL), np.float32)
    xpad[:, HL:HL + SR_T] = xf
    A = xpad.reshape(NTRACKS, -1, PCH)            # [4, 1054, 128]
    Xf = np.ascontiguousarray(A.transpose(2, 1, 0))     # [128, 1054, 4]
    NR = 784                                       # reversed cols needed <= 783
    Xr = np.zeros((128, NR, 4), np.float32)
    Xr[0] = A[:, 1:NR + 1, 0].transpose(1, 0)      # p=0: xpad[(j+1)*128]
    Xr[1:] = np.flip(A[:, :NR, 1:], axis=2).transpose(2, 1, 0)

    krF = kr[:, HL:].copy()
    krF[:, 0] *= 0.5                               # s[n,0] double-counts center
    kiF = ki[:, HL:]

    in_maps = []
    for q in range(NCORES):
        cA0, cB0, a_m1 = _core_assign(q)
        fa = _take_cols(Xf, 271 + cA0, FCA)
        fb = _take_cols(Xf, 271 + cB0, FCB)
        ra = _take_cols(Xr, 270 - cA0 - (NA - 1), FCA)
        rb = _take_cols(Xr, 270 - cB0 - (NB - 1), FCB)
        xarr = np.ascontiguousarray(
            np.concatenate([fa, fb, ra, -ra, rb, -rb], axis=1).astype(bf16)
        )

        kt_all = np.zeros((NG, 128, 4 * 256), np.float32)
        for s in range(NSLOTS):
            m1 = a_m1 and s < NA
            c = cA0 + s if s < NA else cB0 + (s - NA)
            if c * PCH >= HL:
                continue                           # pad slot: zero weights
            rows = slice(128, KBINS) if m1 else slice(0, 128)
            g, si = _slot_group(s)
            for part, mat in ((0, krF), (1, kiF)):
                blk = mat[rows, c * PCH:(c + 1) * PCH]     # [nbins, 128]
                buf = np.zeros((128, PCH), np.float32)
                buf[: blk.shape[0]] = blk
                kt_all[g, :, si * 256 + part * 128: si * 256 + (part + 1) * 128] = buf.T
        in_maps.append({
            "x": xarr,
            "kt": np.ascontiguousarray(kt_all.astype(bf16)),
        })
    return in_maps


def _combine(outs):
    re_acc = np.zeros((KBINS, NTRACKS, NF), np.float32)
    im_acc = np.zeros((KBINS, NTRACKS, NF), np.float32)
    for q in range(NCORES):
        _, _, a_m1 = _core_assign(q)
        out = np.asarray(outs[q], dtype=np.float32)   # [2, 128, 4*258]
        for b in range(8):
            cls, part, tp = b >> 2, (b >> 1) & 1, b & 1
            m1 = a_m1 and cls == 0
            arr = out[cls, :, (b & 3) * N2:((b & 3) + 1) * N2].reshape(128, NF, 2)
            rows = slice(128, KBINS) if m1 else slice(0, 128)
            nrows = KBINS - 128 if m1 else 128
            tgt = re_acc if part == 0 else im_acc
            tgt[rows, tp * 2:(tp + 1) * 2] += arr[:nrows].transpose(0, 2, 1)
    y = np.sqrt(re_acc**2 + im_acc**2)                # [252, 4, 129]
    return np.ascontiguousarray(
        y.reshape(KBINS, SR_B, SR_TR, NF).transpose(1, 0, 3, 2)
    )


def kernel(x, kr, ki):
    global _PROG
    from concourse.bass_utils import run_bass_kernel_spmd

    if _PROG is None:
        _PROG = _build_program()
    in_maps = _pack_inputs(x, kr, ki)
    res = run_bass_kernel_spmd(_PROG, in_maps, core_ids=list(range(NCORES)))
    outs = [res.results[q]["out"] for q in range(NCORES)]
    return _combine(outs)
